# revision 4
# baseline (speedup 1.0000x reference)
"""V13: ragged segment mean via exact gather, with the DMA ramp filled by
prefix copies, per-group call plans, and a small tapered tail group.

Structure (per core; 8 cores data-parallel over B):
- Exact ragged gather of the needed seq rows via gpsimd.dma_gather
  (int16 idx in 16-partition wrap, replicated on-chip to 128 partitions
  through an is_equal-selection matmul routed via PSUM). Bus cost is
  rows x 2KiB at the 360GB/s model ceiling: ~188us for ~33.1k rows.
- Per row-tile, a [128,64] selection matrix (DVE one tensor_scalar) and
  a TensorE f32r matmul accumulate psum[slots, 512]; per group one DVE
  flush + SP store. Host divides by segment length on the way out.
- Ramp (was 4,982ns idle in V9, now ~2,000): SP issues gidx0 (the first
  192 idx columns, shipped pre-replicated so gather calls 0-2 never wait
  on on-chip replication), then ONE fused prefix DMACopy of rows
  [128,256) of slots 0..5 (data-independent addresses; the host parks
  slots with begin<=128<=256<=end there, and those 768 rows leave the
  gather stream), then i16raw. The idx-replication converts/copy-backs
  run on the otherwise-idle Activation engine (the scheduler demotes
  them behind the sel stream on DVE, stalling gather call 1 by ~10us).
- Groups: GBS=[64,64,64,61,3] slots. Group plans are shared across cores
  (one program): group 0 swap-balanced to an equal multiple-of-128 row
  target on every core so all its calls are static-full (covers the
  first-BUFS boot-NaN window with zero padding); groups 1-2 hit equal
  targets; groups 3-4 absorb the per-core spread in dynamic last calls.
- Tail (was 6,032ns after the last gather byte, now ~4,990): the last
  group holds 3 long slots (~720 rows) whose [..,2,1,1]-tapered stream
  covers group 3''s compute/flush/store drain; after the final 1-tile
  call lands only sem + one hot matmul + one DVE flush + a 3-row (6KB)
  store chain remain. Group 3 is tapered [..,2,1,1] as well.
- colw ships int16 (half bus bytes) and converts once on Activation.

Tried and rejected (TimelineSim evidence): flush split across DVE+Act
(Act receives the psum-ready sem ~890ns late), flush/copies on gpsimd
(Pool queue serializes with gather generation), prepared
dma_scatter_add + trigger_dma for the tail store (TimelineSim no_exec
never applies the prep''s DMASW completion tick for user-sem preps ->
simulated deadlock), tapering into the PE p-state (cold 788ns matmuls
land on the critical flush chain).

TimelineSim (worst core): 201,545 ns vs 205,349 (V9 baseline); gather
bus floor ~195,000 + ~2,000 ramp + ~4,990 tail.
"""

import time

import numpy as np

import concourse.bass as bass
from concourse import bacc
import concourse.mybir as mybir
import concourse.tile as tile
from concourse.bass_utils import run_bass_kernel_spmd

B, L, D = 2048, 512, 512
NCORES = 8
BL = B // NCORES  # 256
GB = 64  # slots per group (region = GB*L = 32768 rows, int16 idx max)
CT = 8  # tiles per full dma_gather call (8*128=1024 idx = SWDGE ring cap)
GRPS = BL // GB  # 4 groups per core
NPFX = 6  # prefix HWDGE copies (slots 0..NPFX-1 of group 0, rows [128,256))
PFX_LO, PFX_HI = 128, 256
GCOLS0 = 192  # idx columns shipped pre-replicated (covers gather calls 0-2,
# so the first gathers never wait on the on-chip idx replication)
BUFS = 7  # gtile pool depth; first BUFS gather calls must be fully written

_CACHE = {}
LAST_RESULTS = None
LAST_SPMD = None
STATIC_CNTS = None  # tlsim-only: per-call static num_idxs specialization
RACE_CHECK = True


def _adjust_group(gslots, grows, lens, g, target, lo):
    """Greedy swap search: move group g's row sum to `target` by swapping
    slots (index >= lo protects pinned prefix slots) with later groups.
    Best-effort: stops when no improving swap exists."""
    for _ in range(64):
        diff = target - grows[g]  # want to ADD diff rows to group g
        if diff == 0:
            return True
        best = None
        for k0 in range(lo, len(gslots[g])):
            l0 = int(lens[gslots[g][k0]])
            for g1 in range(g + 1, GRPS):
                for k1 in range(len(gslots[g1])):
                    d = int(lens[gslots[g1][k1]]) - l0
                    gain = abs(diff) - abs(diff - d)
                    if gain > 0 and (best is None or gain > best[0]):
                        best = (gain, k0, g1, k1)
        if best is None:
            return False
        _, k0, g1, k1 = best
        i0, i1 = gslots[g][k0], gslots[g1][k1]
        gslots[g][k0], gslots[g1][k1] = i1, i0
        delta = int(lens[i1]) - int(lens[i0])
        grows[g] += delta
        grows[g1] -= delta
    return grows[g] == target


def _balanced_assignment(length, begin_end):
    """Assign b's to cores (serpentine over descending length) and, within
    each core, to the 4 groups with row sums matched across cores:
    group 0 (holding the NPFX prefix slots, their first 128 rows excluded)
    hits a shared multiple-of-128 target; groups 1-2 hit shared targets;
    group 3 absorbs the per-core spread. Returns asm [NCORES, BL] (slot
    order per core, group-major) and gather_rows [NCORES, GRPS]."""
    begin, end = begin_end
    order = np.argsort(-length, kind="stable")
    cores = np.empty((NCORES, BL), dtype=np.int64)
    for r in range(BL):
        idxs = range(NCORES) if r % 2 == 0 else range(NCORES - 1, -1, -1)
        for j, c in enumerate(idxs):
            cores[c, r] = order[r * NCORES + j]

    per_core = []
    for c in range(NCORES):
        bs = cores[c]
        lens = length[bs]
        qual = np.where((begin[bs] <= PFX_LO) & (end[bs] >= PFX_HI))[0]
        assert len(qual) >= NPFX, (c, len(qual))
        pfx = list(qual[np.argsort(-lens[qual], kind="stable")][:NPFX])
        pfx_set = set(pfx)
        rest = [i for i in range(BL) if i not in pfx_set]
        rest.sort(key=lambda i: -int(lens[i]))
        grows = [int(lens[pfx].sum()) - 128 * NPFX, 0, 0, 0]
        nslots = [NPFX, 0, 0, 0]
        gslots = [list(pfx), [], [], []]
        for i in rest:
            g = min(
                (g for g in range(GRPS) if nslots[g] < GB),
                key=lambda g: grows[g],
            )
            gslots[g].append(i)
            nslots[g] += 1
            grows[g] += int(lens[i])
        per_core.append((bs, lens, gslots, grows))

    g0_mean = np.mean([pc[3][0] for pc in per_core])
    r0 = max(128, int(round(g0_mean / 128)) * 128)
    r1 = int(round(np.mean([pc[3][1] for pc in per_core])))
    r2 = int(round(np.mean([pc[3][2] for pc in per_core])))

    asm = np.empty((NCORES, BL), dtype=np.int64)
    gather_rows = np.zeros((NCORES, GRPS), dtype=np.int64)
    for c, (bs, lens, gslots, grows) in enumerate(per_core):
        _adjust_group(gslots, grows, lens, 0, r0, NPFX)
        _adjust_group(gslots, grows, lens, 1, r1, 0)
        _adjust_group(gslots, grows, lens, 2, r2, 0)
        for g in range(GRPS):
            assert len(gslots[g]) == GB
            asm[c, g * GB : (g + 1) * GB] = bs[gslots[g]]
            gather_rows[c, g] = grows[g]
    return asm, gather_rows


def _group_plan(rows_max, rows_min, taper):
    """Call sizes (in 128-row tiles) for one group sized for rows_max;
    calls are static-full only when every core fills them (rows_min).
    taper=True ends the plan [..., 2, 1] for a short tail chain."""
    tiles = max(-(-rows_max // 128), 1)
    if taper and tiles >= 4:
        base = tiles - 3
        ctiles = [CT] * (base // CT)
        if base % CT:
            ctiles.append(base % CT)
        ctiles += [1, 1, 1]
    else:
        ctiles = [CT] * (tiles // CT)
        if tiles % CT:
            ctiles.append(tiles % CT)
    offs = np.cumsum([ct * 128 for ct in ctiles])
    static_full = [int(o) <= rows_min for o in offs]
    return ctiles, static_full


def _plan_calls(gather_rows):
    """Shared per-group plans from the per-core row matrix."""
    return [
        _group_plan(
            int(gather_rows[:, g].max()),
            int(gather_rows[:, g].min()),
            taper=(g == GRPS - 1),
        )
        for g in range(GRPS)
    ]


def _host_prep(begin_c, end_c, plans):
    """Per-core inputs: compacted per-group gather idx, per-call counts,
    per-tile colw (NPFX prefix cols first, then per-group tile cols)."""
    length = (end_c - begin_c).astype(np.int64)
    nts = [sum(p[0]) for p in plans]
    ncalls = [len(p[0]) for p in plans]
    tot_calls = sum(ncalls)
    tot_nt = sum(nts)
    idx_chunks = []
    colidx = np.full((NPFX + tot_nt, 128), -1.0, dtype=np.float32)
    colidx[:NPFX, :] = np.arange(NPFX, dtype=np.float32)[:, None]
    cnt = np.zeros(tot_calls, dtype=np.int32)
    call_base = 0
    nt_base = NPFX
    for grp in range(GRPS):
        ctiles, static_full = plans[grp]
        nt = nts[grp]
        rows_cap = nt * 128
        offs = np.concatenate([[0], np.cumsum([ct * 128 for ct in ctiles])])
        idx_g = np.zeros((rows_cap,), dtype=np.int64)  # pad = row 0
        ls_parts = []
        slot_parts = []
        for s in range(GB):
            b = grp * GB + s
            if grp == 0 and s < NPFX:
                assert begin_c[b] <= PFX_LO and end_c[b] >= PFX_HI
                rng = np.concatenate(
                    [
                        np.arange(begin_c[b], PFX_LO),
                        np.arange(PFX_HI, end_c[b]),
                    ]
                )
            else:
                rng = np.arange(begin_c[b], end_c[b])
            ls_parts.append(rng)
            slot_parts.append(np.full(len(rng), s, dtype=np.int64))
        ls = np.concatenate(ls_parts)
        slots = np.concatenate(slot_parts)
        n_rows = len(ls)
        assert n_rows <= rows_cap, (grp, n_rows, rows_cap)
        idx_g[:n_rows] = slots * L + ls
        tiles = np.arange(n_rows) // 128
        pos = np.arange(n_rows) % 128
        colidx[nt_base + tiles, pos] = slots.astype(np.float32)
        for call in range(len(ctiles)):
            cap = ctiles[call] * 128
            col = min(max(n_rows - int(offs[call]), 0), cap)
            g = call_base + call
            if static_full[call] or g < BUFS:
                # static-count calls always transfer full capacity; the
                # first BUFS gather slots must also be fully written on
                # first use (boot NaN guard). Padding rows gather row 0
                # and have colidx -1 -> zero selection.
                col = cap
            elif col == 0:
                col = 1  # avoid fully-empty calls (sim chokes)
            cnt[g] = col
        idx_chunks.append(idx_g)
        call_base += len(ctiles)
        nt_base += nt
    idx_all = np.concatenate(idx_chunks)
    assert idx_all.max() < GB * L
    idx16 = idx_all.astype(np.int16).reshape(-1, 16).T  # [16, total/16]
    gidx0 = np.ascontiguousarray(np.tile(idx16[:, :GCOLS0], (8, 1)))
    gidxr = np.ascontiguousarray(idx16[:, GCOLS0:])
    colw = np.ascontiguousarray(colidx.T)  # [128, NPFX + tot_nt]
    cnt2 = cnt.reshape(1, tot_calls)
    return (colw, gidx0, gidxr, np.ascontiguousarray(cnt2))


def _build_bass(plans):
    """plans: per-group (ctiles, static_full), shared by all cores."""
    nc = bacc.Bacc("TRN2", detect_race_conditions=RACE_CHECK)
    f32 = mybir.dt.float32
    i32 = mybir.dt.int32
    f32r = mybir.dt.float32r
    nts = [sum(p[0]) for p in plans]
    ncalls = [len(p[0]) for p in plans]
    tot_calls = sum(ncalls)
    tot_nt = sum(nts)
    tcols = tot_nt * 8  # total idx columns (16 idx per column)
    seq = nc.dram_tensor("seq", [BL, L, D], f32r, kind="ExternalInput")
    colw = nc.dram_tensor("colw", [128, NPFX + tot_nt], f32,
                          kind="ExternalInput")
    gidx0 = nc.dram_tensor("gidx0", [128, GCOLS0], mybir.dt.int16,
                           kind="ExternalInput")
    gidxr = nc.dram_tensor("gidxr", [16, tcols - GCOLS0], mybir.dt.int16,
                           kind="ExternalInput")
    gcnt = nc.dram_tensor("gcnt", [1, tot_calls], i32, kind="ExternalInput")
    outn = nc.dram_tensor("outn", [BL, D], f32, kind="ExternalOutput")

    rows = seq[:].rearrange("b l d -> (b l) d")  # [BL*L, D]

    with tile.TileContext(nc) as tc:
        with (
            tc.tile_pool(name="gpool", bufs=BUFS) as gpool,
            tc.tile_pool(name="selp", bufs=6) as selp,
            tc.tile_pool(name="constp", bufs=1) as constp,
            tc.tile_pool(name="psump", bufs=2, space="PSUM") as psump,
            tc.tile_pool(name="outp", bufs=2) as outp,
        ):
            cnt_sb = constp.tile([1, tot_calls], i32)
            pfxw = constp.tile([128, NPFX], f32, name="pfxw")
            colw_sb = []
            for grp in range(GRPS):
                colw_sb.append(constp.tile([128, nts[grp]], f32,
                                           name=f"colw{grp}"))
            iota_f = constp.tile([128, GB], f32)
            idxall = constp.tile([128, tcols], mybir.dt.int16, name="idxall")
            # SP queue order: gidx0 (gates the first gather's generation),
            # then ONE fused prefix copy (bus busy from ~2.6us while the
            # gather chain resolves), then i16raw (gates idx replication
            # for calls >= 1). colw/cnt ride the Activation queue.
            # SP queue order sets the early bus schedule: gidx0 (91ns, its
            # sem chain overlaps the prefix transfer), the fused prefix
            # copy (4.4us of bus while the first gather's generation
            # resolves), i16raw (gates idx replication), then the
            # non-urgent consts. A second HWDGE queue would steal slots
            # between these and push the prefix copy's bus grant back.
            pfx_all = constp.tile([128, NPFX * D], f32r, name="pfx_all")
            # first prefix slot alone (bus busy from ~1.97us), gidx0 rides
            # the second HWDGE slot (its transfer slips into the gap after
            # slot 0), then the remaining prefix slots
            nc.sync.dma_start(
                out=pfx_all[:, 0 : 2 * D].rearrange(
                    "p (b d) -> p b d", d=D
                ),
                in_=seq[0:2, PFX_LO:PFX_HI, :].rearrange("b l d -> l b d"),
            )
            nc.sync.dma_start(out=idxall[:, 0:GCOLS0], in_=gidx0[:])
            nc.sync.dma_start(
                out=pfx_all[:, 2 * D :].rearrange("p (b d) -> p b d", d=D),
                in_=seq[2:NPFX, PFX_LO:PFX_HI, :].rearrange(
                    "b l d -> l b d"
                ),
            )
            i16raw = constp.tile([16, tcols - GCOLS0], mybir.dt.int16,
                                 name="i16raw")
            nc.sync.dma_start(out=i16raw[:], in_=gidxr[:])
            nc.sync.dma_start(out=pfxw[:], in_=colw[:, 0:NPFX])
            nc.sync.dma_start(out=colw_sb[0][:],
                              in_=colw[:, NPFX : NPFX + nts[0]])
            nc.sync.dma_start(out=cnt_sb[:], in_=gcnt[:])
            nbase = nts[0]
            for grp in range(1, GRPS):
                nc.sync.dma_start(
                    out=colw_sb[grp][:],
                    in_=colw[:, NPFX + nbase : NPFX + nbase + nts[grp]],
                )
                nbase += nts[grp]
            nc.gpsimd.iota(
                out=iota_f[:],
                pattern=[[1, GB]],
                base=0,
                channel_multiplier=0,
                allow_small_or_imprecise_dtypes=True,
            )
            # repl[p, j] = (j % 16 == p): for on-chip idx replication
            jmod = constp.tile([16, 128], f32, name="jmod")
            nc.gpsimd.iota(
                out=jmod[:], pattern=[[0, 8], [1, 16]], base=0,
                channel_multiplier=0, allow_small_or_imprecise_dtypes=True,
            )
            piota = constp.tile([16, 1], f32, name="piota")
            nc.gpsimd.iota(
                out=piota[:], pattern=[[0, 1]], base=0,
                channel_multiplier=1, allow_small_or_imprecise_dtypes=True,
            )
            repl = constp.tile([16, 128], f32, name="repl")
            nc.vector.tensor_scalar(
                out=repl[:], in0=jmod[:], scalar1=piota[:], scalar2=None,
                op0=mybir.AluOpType.is_equal,
            )
            # idx replication chunks run on the otherwise-idle Activation
            # engine: the scheduler demoted them on DVE behind the sel
            # stream, which stalled gather call 1 by ~10us
            conv = constp.tile([16, tcols - GCOLS0], f32, name="conv")
            pos = 0
            while pos < tcols - GCOLS0:
                w = min(512, tcols - GCOLS0 - pos)
                nc.scalar.copy(
                    out=conv[:, pos : pos + w], in_=i16raw[:, pos : pos + w]
                )
                pidx = psump.tile([128, w], f32, tag="pi", name=f"pidx{pos}")
                nc.tensor.matmul(
                    out=pidx[:], lhsT=repl[:], rhs=conv[:, pos : pos + w],
                    start=True, stop=True,
                )
                nc.scalar.copy(
                    out=idxall[:, GCOLS0 + pos : GCOLS0 + pos + w],
                    in_=pidx[:],
                )
                pos += w

            call_base = 0
            cbase_g = 0  # idx column base of this group
            for grp in range(GRPS):
                ctiles, static_full = plans[grp]
                psum = psump.tile([GB, D], f32, tag="ps", name="psum")
                tbase = 0
                cbase = 0
                for call in range(len(ctiles)):
                    ct_c = ctiles[call]
                    if grp == 0 and call == len(ctiles) - 1:
                        # prefix reductions: emitted late so the PE queue's
                        # head stays clear for the idx-replication matmuls
                        # (whose results gate gather calls >= 1); they run
                        # mid-stream in PE idle time, before psum0's stop.
                        assert call > 0
                        for k in range(NPFX):
                            sel = selp.tile([128, GB], f32r, tag="sel",
                                            name="sel")
                            nc.vector.tensor_scalar(
                                out=sel[:],
                                in0=iota_f[:],
                                scalar1=pfxw[:, k : k + 1],
                                scalar2=None,
                                op0=mybir.AluOpType.is_equal,
                            )
                            nc.tensor.matmul(
                                out=psum[:], lhsT=sel[:],
                                rhs=pfx_all[:, k * D : (k + 1) * D],
                                start=False, stop=False,
                            )
                    g = call_base + call
                    gtile = gpool.tile([128, ct_c * D], f32r, tag="g",
                                       name="gtile")
                    if STATIC_CNTS is None:
                        if static_full[call] or g < BUFS:
                            cnt_rv = ct_c * 128
                        else:
                            cnt_rv = nc.gpsimd.value_load(
                                cnt_sb[0:1, g : g + 1]
                            )
                        nc.gpsimd.dma_gather(
                            gtile[:].rearrange("p (c e) -> p c e", e=D),
                            rows[grp * GB * L : (grp + 1) * GB * L, :],
                            idxall[:, cbase_g + cbase
                                   : cbase_g + cbase + ct_c * 8],
                            ct_c * 128,
                            cnt_rv,
                            D,
                        )
                    else:
                        cntv = int(STATIC_CNTS[g])
                        ni = -(-cntv // 16) * 16  # round up to 16
                        nc.gpsimd.dma_gather(
                            gtile[:].rearrange("p (c e) -> p c e", e=D)[
                                :, : -(-ni // 128), :
                            ],
                            rows[grp * GB * L : (grp + 1) * GB * L, :],
                            idxall[:, cbase_g + cbase
                                   : cbase_g + cbase + ni // 16],
                            ni,
                            cntv,
                            D,
                        )
                    for t in range(ct_c):
                        tg = tbase + t  # tile id within group
                        sel = selp.tile([128, GB], f32r, tag="sel",
                                        name="sel")
                        nc.vector.tensor_scalar(
                            out=sel[:],
                            in0=iota_f[:],
                            scalar1=colw_sb[grp][:, tg : tg + 1],
                            scalar2=None,
                            op0=mybir.AluOpType.is_equal,
                        )
                        tile_first = call == 0 and t == 0
                        tile_last = (call == len(ctiles) - 1
                                     and t == ct_c - 1)
                        nc.tensor.matmul(
                            out=psum[:],
                            lhsT=sel[:],
                            rhs=gtile[:, t * D : (t + 1) * D],
                            start=tile_first,
                            stop=tile_last,
                        )
                    tbase += ct_c
                    cbase += ct_c * 8
                out_sb = outp.tile([GB, D], f32, tag="out", name="out_sb")
                nc.vector.tensor_copy(out=out_sb[:], in_=psum[:])
                nc.sync.dma_start(
                    out=outn[grp * GB : (grp + 1) * GB, :], in_=out_sb[:]
                )
                call_base += len(ctiles)
                cbase_g += nts[grp] * 8
    nc.compile()
    return nc


def _plan_key(plans):
    return tuple((tuple(ct), tuple(sf)) for ct, sf in plans)


def _get_bass(plans):
    key = ("nc", _plan_key(plans))
    if key not in _CACHE:
        _CACHE[key] = _build_bass(plans)
    return _CACHE[key]


def kernel(seq, begin, end):
    global LAST_RESULTS, LAST_SPMD
    seq = np.ascontiguousarray(np.asarray(seq, dtype=np.float32))
    begin_i = np.asarray(begin).astype(np.int64)
    end_i = np.asarray(end).astype(np.int64)
    length = end_i - begin_i
    asm, gather_rows = _balanced_assignment(length, (begin_i, end_i))
    plans = _plan_calls(gather_rows)

    nc = _get_bass(plans)
    in_maps = []
    for c in range(NCORES):
        bs = asm[c]
        colw, gidx0, gidxr, cnt = _host_prep(begin_i[bs], end_i[bs], plans)
        in_maps.append({"seq": seq[bs], "colw": colw, "gidx0": gidx0,
                        "gidxr": gidxr, "gcnt": cnt})

    LAST_SPMD = (nc, in_maps)
    # the axon-tunneled devices occasionally report a transient
    # NRT_EXEC_UNIT_UNRECOVERABLE; a fresh attempt recovers
    last_exc = None
    for attempt in range(3):
        try:
            LAST_RESULTS = run_bass_kernel_spmd(
                nc, in_maps, core_ids=list(range(NCORES))
            )
            break
        except Exception as e:  # noqa: BLE001
            last_exc = e
            time.sleep(10.0)
    else:
        raise last_exc
    out = np.empty((B, D), dtype=np.float32)
    w_all = (1.0 / length.astype(np.float32)).astype(np.float32)
    for c in range(NCORES):
        out[asm[c]] = (LAST_RESULTS.results[c]["outn"]
                       * w_all[asm[c]][:, None])
    return out
